# revision 1
# baseline (speedup 1.0000x reference)
"""ArcFace loss on 8 TRN2 NeuronCores (Bass/Tile, class-sharded classifier).

Math: since cos(arccos(clip(x))) == clip(x), non-target logits are just
SCALE*clip(cosine). Only the target-class logit needs the margin:
l' = SCALE*(x*cos(m) - sqrt(1-x^2)*sin(m)).  Logits are <= SCALE, so
logsumexp uses the fixed max SCALE=64: lse = 64 + log(sum exp(l-64)).
Each core owns C/8 = 3750 classes (padded to 3840 with zero rows),
computes partial sum_c exp(64*clip(cos)-64) for all 4096 rows, plus the
target-row dot/norm for labels it owns; one 48KB AllReduce combines
[S, t, q] and every core finishes the (tiny) scalar loss identically.
"""

import math
import os
import sys
import types

import numpy as np

import concourse.bass as bass
import concourse.mybir as mybir
import concourse.tile as tile
from concourse import bacc
from concourse.bass_utils import run_bass_kernel_spmd
from concourse.masks import make_identity


def _install_profile_hook():
    """Make BASS_TRACE=1 work under axon images whose antenv lacks
    axon_hooks: register a sys.modules shim + the ctypes NTFF hook."""
    try:
        import antenv.axon_hooks  # noqa: F401
        return
    except ImportError:
        pass
    holder = {"fn": None}
    mod = types.ModuleType("antenv.axon_hooks")
    mod.set_axon_ntff_profile_hook = lambda fn: holder.__setitem__("fn", fn)
    mod.get_axon_ntff_profile_hook = lambda: holder["fn"]
    sys.modules["antenv.axon_hooks"] = mod
    try:
        import antenv
        antenv.axon_hooks = mod
    except ImportError:
        pass
    try:
        from trn_agent_boot.trn_boot import _ntff_profile_via_ctypes
        so = "/opt/axon/libaxon_pjrt.so"
        if os.path.exists(so):
            mod.set_axon_ntff_profile_hook(_ntff_profile_via_ctypes(so))
    except Exception:
        pass


_install_profile_hook()

F32 = mybir.dt.float32
BF16 = mybir.dt.bfloat16
I32 = mybir.dt.int32

N, D, C = 4096, 512, 30000
NCORES = 8
CREAL = C // NCORES          # 3750 real classes per core
CS = 4096                    # padded shard rows (32 * 128, symmetric with N)
DUMMY = CREAL                # a guaranteed-zero row in every shard
NB = N // 128                # 32 n-blocks
CB = CS // 128               # 32 c-blocks
DCH = D // 128               # 4 contraction chunks
CCH = 512                    # matmul chunk (one psum bank of f32)
NCC = CS // CCH              # 8 c-chunks
NGRP = 4                     # transpose / availability groups
GR = NB // NGRP              # 8 row-blocks per group
SCALE = 64.0
MARGIN = 0.5
COS_M = math.cos(MARGIN)
SIN_M = math.sin(MARGIN)
HI = 1.0 - 1e-7              # upper cosine clip (reference semantics)
LO_BULK = -0.35              # lower clip: exp(64*-0.35-64)=e^-86.4 ~ 1e-38 ~ 0
LO_TGT = -1.0 + 1e-7         # exact lower clip for the target-class formula

AX = mybir.AluOpType
AF = mybir.ActivationFunctionType

LAST_RESULT = None           # test.py reads exec_time_ns from here


def _build():
    nc = bacc.Bacc("TRN2", target_bir_lowering=False, debug=False,
                   num_devices=NCORES)

    emb = nc.dram_tensor("emb", [N, D], F32, kind="ExternalInput")
    wsh = nc.dram_tensor("wsh", [CS, D], F32, kind="ExternalInput")
    lab = nc.dram_tensor("lab", [128, NB], I32, kind="ExternalInput")
    out = nc.dram_tensor("out", [1, 1], F32, kind="ExternalOutput")

    with tile.TileContext(nc) as tc:
        with (
            tc.tile_pool(name="pers", bufs=1) as pers,
            tc.tile_pool(name="strm", bufs=4) as strm,
            tc.tile_pool(name="evac", bufs=4) as evac,
            tc.tile_pool(name="ppmm", bufs=3, space="PSUM") as ppmm,
            tc.tile_pool(name="ppfin", bufs=1, space="PSUM") as ppfin,
            tc.tile_pool(name="dram", bufs=1, space="DRAM") as dram,
        ):
            # ---- constants / persistent state ----
            ones_col = pers.tile([128, 1], F32)
            nc.vector.memset(ones_col[:], 1.0)
            bias_m64 = pers.tile([128, 1], F32)
            nc.vector.memset(bias_m64[:], -SCALE)
            bias_p64 = pers.tile([128, 1], F32)
            nc.vector.memset(bias_p64[:], SCALE)

            lab_sb = pers.tile([128, NB], I32)
            nc.sync.dma_start(lab_sb[:], lab[:])

            # bf16 staging in DRAM, d-major so XBAR transpose reads are
            # contiguous
            e_stage = dram.tile([DCH, N, 128], BF16)
            w_stage = dram.tile([DCH, CS, 128], BF16)
            eT = [pers.tile([128, N], BF16, name=f"eT{d}") for d in range(DCH)]
            wT = [pers.tile([128, CS], BF16, name=f"wT{d}") for d in range(DCH)]

            nrm2 = pers.tile([128, CB], F32)
            nrm = pers.tile([128, CB], F32)
            rn = pers.tile([128, CB], F32)
            t_sb = pers.tile([128, NB], F32)
            q_sb = pers.tile([128, NB], F32)
            scols = pers.tile([128, NB * NGRP], F32)

            # deferred DVE reductions (sprinkled through the main loop so
            # they never head-of-line-block the evac work)
            pending = []
            wnat_tiles = {}
            cc1_state = {"done": False}

            def emit_pending(n_ops):
                for _ in range(min(n_ops, len(pending))):
                    pending.pop(0)()

            def main_unit(cg, nb):
                tlo = cg * 2 * CCH
                ps_cos = ppmm.tile([128, 2 * CCH], F32, name="ps_cos")
                for h in range(2):
                    for d in range(DCH):
                        nc.tensor.matmul(
                            ps_cos[:, h * CCH:(h + 1) * CCH],
                            lhsT=eT[d][:, nb * 128:(nb + 1) * 128],
                            rhs=wT[d][:, tlo + h * CCH:tlo + (h + 1) * CCH],
                            start=(d == 0), stop=(d == DCH - 1))
                exp_t = evac.tile([128, 2 * CCH], BF16, name="exp_t")
                nc.scalar.activation(exp_t[:], ps_cos[:], AF.Exp,
                                     bias=bias_m64[:], scale=SCALE)
                mn_t = evac.tile([128, 2 * CCH], BF16, name="mn_t")
                col = nb * NGRP + cg
                nc.vector.tensor_scalar(
                    out=mn_t[:], in0=exp_t[:], scalar1=1.0, scalar2=None,
                    op0=AX.min, op1=AX.add,
                    accum_out=scols[:, col:col + 1])

            for g in range(NGRP):
                ks = range(g * GR, (g + 1) * GR)
                for k in ks:
                    # W: load + row-norm^2 (ACT square+accum, one table)
                    w_nat = strm.tile([128, D], F32, name="w_nat", bufs=11)
                    wnat_tiles[k] = w_nat
                    nc.sync.dma_start(w_nat[:], wsh[k * 128:(k + 1) * 128, :])
                    wsq = strm.tile([128, D], F32, name="wsq", bufs=2)
                    nc.scalar.activation(wsq[:], w_nat[:], AF.Square,
                                         accum_out=nrm2[:, k:k + 1])
                    # E: load, gather, elementwise products on gpsimd,
                    # bf16 cast on DVE, d-major stage via sync
                    e_nat = strm.tile([128, D], F32, name="e_nat")
                    nc.sync.dma_start(e_nat[:], emb[k * 128:(k + 1) * 128, :])
                    wy = strm.tile([128, D], F32, name="wy")
                    nc.gpsimd.indirect_dma_start(
                        out=wy[:], out_offset=None, in_=wsh[:],
                        in_offset=bass.IndirectOffsetOnAxis(
                            ap=lab_sb[:, k:k + 1], axis=0),
                    )
                    ew = strm.tile([128, D], F32, name="ew", bufs=10)
                    nc.gpsimd.tensor_mul(ew[:], e_nat[:], wy[:])
                    wy2 = strm.tile([128, D], F32, name="wy2", bufs=10)
                    nc.gpsimd.tensor_mul(wy2[:], wy[:], wy[:])

                    def _red(kk=k, a=ew, b=wy2):
                        nc.vector.reduce_sum(t_sb[:, kk:kk + 1], a[:],
                                             axis=mybir.AxisListType.X)
                        nc.vector.reduce_sum(q_sb[:, kk:kk + 1], b[:],
                                             axis=mybir.AxisListType.X)
                    pending.append(_red)

                    e_bf = strm.tile([128, D], BF16, name="e_bf")
                    nc.vector.tensor_copy(e_bf[:], e_nat[:])
                    for d in range(DCH):
                        nc.sync.dma_start(
                            e_stage[d, k * 128:(k + 1) * 128, :],
                            e_bf[:, d * 128:(d + 1) * 128])
                # batched 1/||w|| for the group (one Sqrt table load)
                gs = slice(g * GR, (g + 1) * GR)
                nc.scalar.sqrt(nrm[:, gs], nrm2[:, gs])
                nc.vector.tensor_scalar_max(nrm[:, gs], nrm[:, gs], 1e-12)
                nc.vector.reciprocal(rn[:, gs], nrm[:, gs])
                # normalize + store (scalar queue: copy then its own store)
                for k in ks:
                    w_nrm = strm.tile([128, D], BF16, name="w_nrm")
                    nc.scalar.activation(w_nrm[:], wnat_tiles[k][:], AF.Copy,
                                         scale=rn[:, k:k + 1])
                    for d in range(DCH):
                        nc.scalar.dma_start(
                            w_stage[d, k * 128:(k + 1) * 128, :],
                            w_nrm[:, d * 128:(d + 1) * 128])
                # XBAR transposes for this group (sync queue)
                lo, hi = g * GR * 128, (g + 1) * GR * 128
                for d in range(DCH):
                    nc.sync.dma_start_transpose(
                        eT[d][:, lo:hi], e_stage[d, lo:hi, :])
                    nc.sync.dma_start_transpose(
                        wT[d][:, lo:hi], w_stage[d, lo:hi, :])
                # main blocks that just became feasible: cg == g, all ng<=g
                # plus earlier cgs' ng == g blocks
                for cg, ng in [(g, n2) for n2 in range(g + 1)] + \
                              [(c2, g) for c2 in range(g)]:
                    for nb in range(ng * GR, (ng + 1) * GR):
                        main_unit(cg, nb)
                        emit_pending(1 if len(pending) < 12 else 2)
                    if (g == NGRP - 1 and not pending
                            and not cc1_state["done"]):
                        cc1_state["done"] = True
                        cc1_in = dram.tile([2, 128, NB], F32)
                        cc1_out = dram.tile([2, 128, NB], F32)
                        nc.sync.dma_start(cc1_in[0], t_sb[:])
                        nc.sync.dma_start(cc1_in[1], q_sb[:])
                        nc.gpsimd.collective_compute(
                            "AllReduce", AX.add,
                            replica_groups=[list(range(NCORES))],
                            ins=[cc1_in[:]], outs=[cc1_out[:]])
                        t_tot = pers.tile([128, NB], F32)
                        q_tot = pers.tile([128, NB], F32)
                        nc.sync.dma_start(t_tot[:], cc1_out[0])
                        nc.sync.dma_start(q_tot[:], cc1_out[1])
            assert cc1_state["done"] and not pending

            # ---- collective #2: softmax partial sums ----
            s_n = pers.tile([128, NB], F32)
            nc.vector.reduce_sum(
                s_n[:],
                scols[:].rearrange("p (nb c) -> p nb c", c=NGRP),
                axis=mybir.AxisListType.X)
            cc2_in = dram.tile([128, NB], F32)
            cc2_out = dram.tile([128, NB], F32)
            nc.sync.dma_start(cc2_in[:], s_n[:])
            nc.gpsimd.collective_compute(
                "AllReduce", AX.add,
                replica_groups=[list(range(NCORES))],
                ins=[cc2_in[:]], outs=[cc2_out[:]])
            s_tot = pers.tile([128, NB], F32)
            nc.sync.dma_start(s_tot[:], cc2_out[:])

            # ---- final scalar loss (tail; replicated on every core) ----
            ny = pers.tile([128, NB], F32)
            nc.scalar.sqrt(ny[:], q_tot[:])
            nc.vector.tensor_scalar_max(ny[:], ny[:], 1e-12)
            rny = pers.tile([128, NB], F32)
            nc.vector.reciprocal(rny[:], ny[:])
            x = pers.tile([128, NB], F32)
            nc.vector.tensor_mul(x[:], t_tot[:], rny[:])       # cos_y
            nc.vector.tensor_scalar(out=x[:], in0=x[:], scalar1=HI,
                                    scalar2=LO_TGT, op0=AX.min, op1=AX.max)
            xsq = pers.tile([128, NB], F32)
            nc.scalar.square(xsq[:], x[:])
            s1mx = pers.tile([128, NB], F32)                   # sqrt(1-x^2)
            nc.scalar.activation(s1mx[:], xsq[:], AF.Sqrt, bias=1.0,
                                 scale=-1.0)
            lm = pers.tile([128, NB], F32)                     # margin logit
            nc.vector.tensor_scalar_mul(lm[:], x[:], SCALE * COS_M)
            sb_t = pers.tile([128, NB], F32)
            nc.vector.tensor_scalar_mul(sb_t[:], s1mx[:], SCALE * SIN_M)
            nc.vector.tensor_sub(lm[:], lm[:], sb_t[:])
            sub_t = pers.tile([128, NB], F32)
            nc.scalar.activation(sub_t[:], x[:], AF.Exp, bias=bias_m64[:],
                                 scale=SCALE)
            add_t = pers.tile([128, NB], F32)
            nc.scalar.activation(add_t[:], lm[:], AF.Exp, bias=bias_m64[:],
                                 scale=1.0)
            sadj = pers.tile([128, NB], F32)
            nc.vector.tensor_sub(sadj[:], s_tot[:], sub_t[:])
            nc.vector.tensor_add(sadj[:], sadj[:], add_t[:])
            lse = pers.tile([128, NB], F32)
            nc.scalar.activation(lse[:], sadj[:], AF.Ln)
            lossn = pers.tile([128, NB], F32)                  # loss - 64
            nc.vector.tensor_sub(lossn[:], lse[:], lm[:])
            red1 = pers.tile([128, 1], F32)
            nc.vector.reduce_sum(red1[:], lossn[:], axis=mybir.AxisListType.X)
            ps_fin = ppfin.tile([1, 1], F32, name="ps_fin")
            nc.tensor.matmul(ps_fin[:], lhsT=ones_col[:], rhs=red1[:],
                             start=True, stop=True)
            out_sb = pers.tile([1, 1], F32)
            nc.scalar.activation(out_sb[:], ps_fin[:], AF.Identity,
                                 bias=bias_p64[:1, :1], scale=1.0 / N)
            nc.sync.dma_start(out[:1, :1], out_sb[:])

    nc.finalize()
    return nc


_NC = None


def _get_nc():
    global _NC
    if _NC is None:
        _NC = _build()
    return _NC


def kernel(embeddings, labels, classifier_weights):
    global LAST_RESULT
    E = np.ascontiguousarray(np.asarray(embeddings, dtype=np.float32))
    W = np.ascontiguousarray(np.asarray(classifier_weights, dtype=np.float32))
    lab = np.asarray(labels).astype(np.int64).ravel()
    assert E.shape == (N, D) and W.shape == (C, D) and lab.shape == (N,)

    nc = _get_nc()
    in_maps = []
    for i in range(NCORES):
        lo, hi = i * CREAL, (i + 1) * CREAL
        wsh_i = np.zeros((CS, D), dtype=np.float32)
        wsh_i[:CREAL] = W[lo:hi]
        loc = np.where((lab >= lo) & (lab < hi), lab - lo, DUMMY)
        lab_i = np.ascontiguousarray(
            loc.reshape(NB, 128).T.astype(np.int32))  # [128, NB], n = nb*128+p
        in_maps.append({"emb": E, "wsh": wsh_i, "lab": lab_i})

    res = run_bass_kernel_spmd(nc, in_maps, core_ids=list(range(NCORES)))
    LAST_RESULT = res
    val = np.float32(res.results[0]["out"].reshape(())[()])
    return np.asarray(val, dtype=np.float32).reshape(())

